# revision 13
# baseline (speedup 1.0000x reference)
"""Multi-head cross-attention (B=4, S=2048, D=1024, H=16) on 8 Trainium2 cores.

Sharding: hybrid data/tensor parallel. Core c handles batch b = c//2 and
head-group g = c%2 (8 of the 16 heads, i.e. 512 of the 1024 q/k/v dims).
Each core computes a partial out-projection over its 512 attention dims;
the host sums the two partials per batch (the "all-reduce after out_lin"
of the tensor-parallel split, done on host since pairs share a batch).

Per-core kernel (all matmuls in float32r = rounded-fp32 PE fast path):
  Q.T = wq_t.T @ x_t   (+bq)      [512, 2048]   (o on partitions)
  K.T = wk_t.T @ mem_t (+bk)      [512, 2048]
  V   = mem_t.T @ wv_t            [2048, 512] stored as v_aug [*, 8, 65]
                                  with a ones column per head (denominator)
  per head h, query-half qh:
    S.T[k,q] = K_h @ Q_h.T                (K=64 contraction)
    P.T      = exp(0.125*S.T + mask_bias) (ACT, bias is per-key partition)
    AV.T     = [V_h|1].T @ P.T  -> [65, 1024] PSUM accum over k-chunks
    attn.T   = AV.T[0:64] * recip(AV.T[64])  (Newton-refined reciprocal)
  out.T = wo_t.T @ attn.T (+bo_eff on core g=0)  [1024, 2048] partial

bv is folded into bo on the host: out = attn@wo.T + (bo + wo@bv) because
softmax rows sum to 1. The key-padding mask enters as an additive
per-partition bias in the exp activation (exact, and free).
"""

import numpy as np

import concourse.bacc as bacc
import concourse.mybir as mybir
from concourse import tile
from concourse.bass_utils import run_bass_kernel_spmd

F32 = mybir.dt.float32
F32R = mybir.dt.float32r
F16 = mybir.dt.float16
AF = mybir.ActivationFunctionType

B, S, D = 4, 2048, 1024
H, HD = 16, 64
NCORES = 8
NH = 8          # heads per core
OD = NH * HD    # 512 attention dims per core
P = 128
NDC = D // P    # 8 d-chunks
NKC = S // P    # 16 key chunks
NEG = -1.0e30

_cache = {}


def _build():
    from contextlib import ExitStack

    nc = bacc.Bacc(None, target_bir_lowering=False, debug=False)

    x_t = nc.dram_tensor("x_t", [D, S], F16, kind="ExternalInput").ap()
    mem_t = nc.dram_tensor("mem_t", [D, S], F16, kind="ExternalInput").ap()
    wq_t = nc.dram_tensor("wq_t", [D, OD], F16, kind="ExternalInput").ap()
    wk_t = nc.dram_tensor("wk_t", [D, OD], F16, kind="ExternalInput").ap()
    wv_t = nc.dram_tensor("wv_t", [D, OD], F16, kind="ExternalInput").ap()
    wo_t = nc.dram_tensor("wo_t", [OD, D], F16, kind="ExternalInput").ap()
    bq_s = nc.dram_tensor("bq_s", [P, OD // P], F32, kind="ExternalInput").ap()
    bk_s = nc.dram_tensor("bk_s", [P, OD // P], F32, kind="ExternalInput").ap()
    bo_s = nc.dram_tensor("bo_s", [P, D // P], F32, kind="ExternalInput").ap()
    maskb = nc.dram_tensor("maskb", [P, NKC], F32, kind="ExternalInput").ap()
    out_t = nc.dram_tensor("out_t", [D, S], F32, kind="ExternalOutput").ap()

    x_c = x_t.rearrange("(c p) s -> c p s", p=P)
    m_c = mem_t.rearrange("(c p) s -> c p s", p=P)
    wq_c = wq_t.rearrange("(c p) o -> c p o", p=P)
    wk_c = wk_t.rearrange("(c p) o -> c p o", p=P)
    wv_c = wv_t.rearrange("(c p) o -> c p o", p=P)
    wo_c = wo_t.rearrange("(c p) o -> c p o", p=P)

    with tile.TileContext(nc) as tc, ExitStack() as ctx:
        q_pool = ctx.enter_context(tc.tile_pool(name="qt", bufs=1))
        k_pool = ctx.enter_context(tc.tile_pool(name="kt", bufs=1))
        v_pool = ctx.enter_context(tc.tile_pool(name="va", bufs=1))
        a_pool = ctx.enter_context(tc.tile_pool(name="at", bufs=1))
        c_pool = ctx.enter_context(tc.tile_pool(name="cst", bufs=1))
        w_pool = ctx.enter_context(tc.tile_pool(name="wt", bufs=10))
        e_pool = ctx.enter_context(tc.tile_pool(name="es", bufs=2))
        n_pool = ctx.enter_context(tc.tile_pool(name="nrm", bufs=1))
        o_pool = ctx.enter_context(tc.tile_pool(name="ev", bufs=2))
        psum_pool = ctx.enter_context(tc.tile_pool(name="ps", bufs=2, space="PSUM"))
        xp_ctx = ExitStack()
        x_pool = xp_ctx.enter_context(tc.tile_pool(name="xp", bufs=8))
        mp_ctx = ExitStack()
        m_pool = mp_ctx.enter_context(tc.tile_pool(name="mp", bufs=8))

        # ---- constants ----
        bq_sb = c_pool.tile([P, OD // P], F32, tag="bq")
        bk_sb = c_pool.tile([P, OD // P], F32, tag="bk")
        bo_sb = c_pool.tile([P, D // P], F32, tag="bo")
        mk_sb = c_pool.tile([P, NKC], F32, tag="mk")
        nc.sync.dma_start(out=bq_sb[:], in_=bq_s[:])
        nc.sync.dma_start(out=bk_sb[:], in_=bk_s[:])
        nc.sync.dma_start(out=bo_sb[:], in_=bo_s[:])
        nc.sync.dma_start(out=mk_sb[:], in_=maskb[:])
        ones_f = c_pool.tile([P, NH], F32, tag="onef")
        nc.vector.memset(ones_f[:], 1.0)
        ones_r = c_pool.tile([P, NH], F16, tag="oner")
        nc.vector.tensor_copy(ones_r[:], ones_f[:])

        # ---- persistent tiles ----
        qT = [q_pool.tile([P, S], F16, tag=f"q{m}", name=f"q{m}")
              for m in range(OD // P)]
        kT = [k_pool.tile([P, S], F16, tag=f"k{h}", name=f"k{h}")
              for h in range(NH)]
        for h in range(NH):
            ro = 64 * (h % 2)
            nc.vector.memset(kT[h][64 - ro:128 - ro, :], 0.0)
        v_aug = [v_pool.tile([P, 9, 65], F16, tag=f"v{st}", name=f"v{st}")
                 for st in range(NKC)]
        for st in range(NKC):
            nc.vector.memset(v_aug[st][:, 8, :], 0.0)
        attn = [a_pool.tile([P, S], F16, tag=f"a{m}", name=f"a{m}")
                for m in range(OD // P)]

        # ---- input loads ----
        m_tiles = []
        for i in range(NDC):
            t = m_pool.tile([P, S], F16, tag="mm", name="mt")
            nc.sync.dma_start(out=t[:], in_=m_c[i])
            m_tiles.append(t)
        x_tiles = []
        for i in range(NDC):
            t = x_pool.tile([P, S], F16, tag="xx", name="xt")
            nc.sync.dma_start(out=t[:], in_=x_c[i])
            x_tiles.append(t)

        def emit_k_evict(m, ps, csl):
            nc.vector.tensor_scalar_add(
                kT[2 * m][0:64, csl], ps[0:64, :], bk_sb[0:64, m:m + 1])
            nc.vector.tensor_scalar_add(
                kT[2 * m + 1][64:128, csl], ps[64:128, :],
                bk_sb[64:128, m:m + 1])

        def emit_q_evict(m, ps, csl):
            nc.vector.tensor_scalar_add(qT[m][:, csl], ps[:], bq_sb[:, m:m + 1])

        def kq_proj(m, psum_tag):
            """Generator: emits K[m] then Q[m] projection in ~2-MM steps."""
            for kind, w_c, srcs, evict in (
                ("k", wk_c, m_tiles, emit_k_evict),
                ("q", wq_c, x_tiles, emit_q_evict),
            ):
                wts = []
                for i in range(NDC):
                    wt = w_pool.tile([P, P], F16, tag="w", name=f"w{kind}t",
                                     bufs=10)
                    nc.sync.dma_start(
                        out=wt[:], in_=w_c[i, :, m * P:(m + 1) * P])
                    wts.append(wt)
                for n in range(2):
                    csl = slice(n * 1024, (n + 1) * 1024)
                    ps = psum_pool.tile([P, 1024], F32, tag=psum_tag,
                                        name=f"ps{kind}",
                                        bufs=1 if psum_tag == "po" else None)
                    for i in range(NDC):
                        for j in range(2):
                            nc.tensor.matmul(
                                ps[:, j * 512:(j + 1) * 512], wts[i][:],
                                srcs[i][:, n * 1024 + j * 512:
                                        n * 1024 + (j + 1) * 512],
                                start=(i == 0), stop=(i == NDC - 1),
                            )
                        yield
                    evict(m, ps, csl)
                    yield

        def outproj(qh, psum_tag):
            """Generator: out.T columns for query-half qh in ~2-MM steps."""
            for m in range(D // P):
                wts = []
                for i in range(OD // P):
                    wt = w_pool.tile([P, P], F16, tag="w", name="wot", bufs=10)
                    nc.sync.dma_start(
                        out=wt[:], in_=wo_c[i, :, m * P:(m + 1) * P])
                    wts.append(wt)
                csl = slice(qh * 1024, (qh + 1) * 1024)
                ps = psum_pool.tile([P, 1024], F32, tag=psum_tag, name="pso",
                                    bufs=1 if psum_tag == "po" else None)
                for i in range(OD // P):
                    for j in range(2):
                        nc.tensor.matmul(
                            ps[:, j * 512:(j + 1) * 512], wts[i][:],
                            attn[i][:, qh * 1024 + j * 512:
                                    qh * 1024 + (j + 1) * 512],
                            start=(i == 0), stop=(i == OD // P - 1),
                        )
                    yield
                ev = o_pool.tile([P, 1024], F32, tag="ev")
                nc.vector.tensor_scalar_add(ev[:], ps[:], bo_sb[:, m:m + 1])
                nc.sync.dma_start(out=out_t[m * P:(m + 1) * P, csl], in_=ev[:])
                yield

        def drain(g):
            if g is not None:
                for _ in g:
                    pass

        # ---- pre-attention: K[0], Q[0], V ----
        drain(kq_proj(0, "lg"))
        wv_tiles = []
        for i in range(NDC):
            wt = w_pool.tile([P, OD], F16, tag="wv", name="wvt", bufs=8)
            nc.sync.dma_start(out=wt[:], in_=wv_c[i])
            wv_tiles.append(wt)
        for st in range(NKC):
            ps = psum_pool.tile([P, 1024], F32, tag="lg", name="psv")
            for i in range(NDC):
                nc.tensor.matmul(
                    ps[:, 0:OD], m_tiles[i][:, st * P:(st + 1) * P],
                    wv_tiles[i][:],
                    start=(i == 0), stop=(i == NDC - 1),
                )
            nc.vector.tensor_copy(
                v_aug[st][:, 0:NH, 0:64],
                ps[:, 0:OD].rearrange("p (h d) -> p h d", h=NH),
            )
            nc.vector.tensor_copy(
                v_aug[st][:, 0:NH, 64:65], ones_r[:].unsqueeze(2))
        mp_ctx.close()

        # ---- attention (qh outer), with interleaved proj/outproj ----
        for qh in range(2):
            for mt in range(OD // P):
                if qh == 0:
                    feeder = kq_proj(mt + 1, "po") if mt < OD // P - 1 else None
                else:
                    feeder = ostream if mt == 0 else feeder
                for h in (2 * mt, 2 * mt + 1):
                    ro = 64 * (h % 2)
                    q_sl = slice(qh * 1024, (qh + 1) * 1024)
                    av = psum_pool.tile([P, 1024], F32, tag="av", name="av",
                                        bufs=1)
                    for kc in range(NKC):
                        lg = psum_pool.tile([P, 1024], F32, tag="lg", name="lg")
                        for j in range(2):
                            nc.tensor.matmul(
                                lg[:, j * 512:(j + 1) * 512],
                                kT[h][:, kc * P:(kc + 1) * P],
                                qT[mt][:, qh * 1024 + j * 512:
                                        qh * 1024 + (j + 1) * 512],
                                start=True, stop=True,
                            )
                        es = e_pool.tile([P, 1024], F16, tag="es")
                        nc.scalar.activation(
                            es[:], lg[:], AF.Exp,
                            bias=mk_sb[:, kc:kc + 1], scale=0.125,
                        )
                        va_flat = v_aug[kc][:].rearrange("p h d -> p (h d)")
                        for j in range(2):
                            nc.tensor.matmul(
                                av[:, j * 512:(j + 1) * 512],
                                va_flat[:, 65 * h:65 * h + 128],
                                es[:, j * 512:(j + 1) * 512],
                                start=(kc == 0), stop=(kc == NKC - 1),
                            )
                        if feeder is not None:
                            next(feeder, None)
                    avs = n_pool.tile([65, 1024], F32, tag="avs")
                    nc.vector.tensor_copy(avs[:], av[0:65, :])
                    r0 = n_pool.tile([1, 1024], F32, tag="r0")
                    bc = n_pool.tile([64, 1024], F32, tag="bc")
                    nc.vector.reciprocal(r0[:], avs[64:65, :])
                    nc.gpsimd.partition_broadcast(bc[:], r0[:])
                    nc.vector.tensor_mul(
                        attn[mt][ro:ro + 64, q_sl], avs[0:64, :], bc[:])
                drain(feeder)
                feeder = None
                if qh == 0 and mt == OD // P - 1:
                    xp_ctx.close()
                    ostream = outproj(0, "po")

        # ---- tail: out-projection for query-half 1 ----
        drain(outproj(1, "po"))

    nc.compile()
    return nc


def _prep_inputs(x, memory, mask, wq, bq, wk, bk, wv, bv, wo, bo):
    f = np.float32
    h = np.float16
    wqT = np.ascontiguousarray(wq.T, dtype=f)
    wkT = np.ascontiguousarray(wk.T, dtype=f)
    wvT = np.ascontiguousarray(wv.T, dtype=f)
    woT = np.ascontiguousarray(wo.T, dtype=f)
    bo_eff = (bo.astype(f) + wo.astype(f) @ bv.astype(f))
    zeros_bo = np.zeros_like(bo_eff)
    in_maps = []
    for c in range(NCORES):
        b, g = divmod(c, 2)
        sl = slice(g * OD, (g + 1) * OD)
        bo_c = bo_eff if g == 0 else zeros_bo
        in_maps.append({
            "x_t": np.ascontiguousarray(x[b].T, dtype=h),
            "mem_t": np.ascontiguousarray(memory[b].T, dtype=h),
            "wq_t": np.ascontiguousarray(wqT[:, sl]).astype(h),
            "wk_t": np.ascontiguousarray(wkT[:, sl]).astype(h),
            "wv_t": np.ascontiguousarray(wvT[:, sl]).astype(h),
            "wo_t": np.ascontiguousarray(woT[sl, :]).astype(h),
            "bq_s": np.ascontiguousarray(bq[sl].astype(f).reshape(OD // P, P).T),
            "bk_s": np.ascontiguousarray(bk[sl].astype(f).reshape(OD // P, P).T),
            "bo_s": np.ascontiguousarray(bo_c.reshape(D // P, P).T),
            "maskb": np.ascontiguousarray(
                np.where(mask[b], np.float32(NEG), np.float32(0.0))
                .astype(f).reshape(NKC, P).T),
        })
    return in_maps


def kernel(x, memory, mask, wq, bq, wk, bk, wv, bv, wo, bo, **run_kwargs):
    x = np.asarray(x, dtype=np.float32)
    memory = np.asarray(memory, dtype=np.float32)
    mask = np.asarray(mask)
    if "nc" not in _cache:
        _cache["nc"] = _build()
    nc = _cache["nc"]
    in_maps = _prep_inputs(x, memory, mask, wq, bq, wk, bk, wv, bv, wo, bo)
    res = run_bass_kernel_spmd(nc, in_maps, list(range(NCORES)), **run_kwargs)
    out = np.empty((B, S, D), dtype=np.float32)
    for b in range(B):
        part = res.results[2 * b]["out_t"] + res.results[2 * b + 1]["out_t"]
        out[b] = part.T
    if run_kwargs:
        _cache["last_results"] = res
    return out
